# revision 13
# baseline (speedup 1.0000x reference)
"""Trainium2 Bass kernel for nn_AttentionBlock_29291676959393.

Computation (per batch b):
  gain = cond_norm[b] @ norm_w.T + 1            [D]
  xn   = x * gain * rsqrt(mean(x^2, -1) + eps)
  q,k,v = split(xn @ qkv_w.T)                   heads of 64
  q,k  = cosine-normalized * sqrt(head_scale)
  out  = softmax(q @ k.T) @ v @ out_w.T + x

Sharding: 8 cores = (batch 0..3) x (query-token half 0..1). Each core runs
all 16 heads for its 1024 query tokens; K/V are computed redundantly by the
two cores sharing a batch (no collectives at all).

Device-side layout choices:
  - Host passes transposed weight/activation views (pure np layout work), so
    the device never transposes anything. The host also rotates the token
    axis of x.T per-core so the core's own query half occupies columns
    0..1023 (attention is invariant to key-token permutation as long as K
    and V share it), letting one NEFF serve both halves.
  - gain folds into the qkv weight (per-partition scale in [d, feat] layout).
  - inv_rms folds into V only (cosine norm makes Q/K invariant to it).
  - scores are computed transposed [k, q]; the softmax denominator rides the
    attn@v matmul as a 65th ones-column of V (M=65); no max-subtraction is
    needed since cosine-sim scores are bounded by head_scale=10.
  - rsqrt/recip computed as exp(-a*ln(x)) on ACT (one table set with Exp).
  - all matmuls bf16 with f32 PSUM accumulation.
  - attention inner loop is software-pipelined at kt-group granularity
    (scores g+1 interleaved with attn@v g) so the in-order PE never sits in
    multi-us exp waits (keeps the HAM clock at 2.4 GHz).
"""

import numpy as np
import ml_dtypes

import concourse.bass as bass
import concourse.bacc as bacc
import concourse.tile as tile
from concourse import mybir
from concourse.bass_utils import run_bass_kernel_spmd

FP = mybir.dt.float32
BF = mybir.dt.bfloat16
AF = mybir.ActivationFunctionType

P = 128
N_B, L, D = 4, 2048, 1024
NH, E = 16, 64
NQ = L // 2          # query tokens per core
EPS = 1e-6
DCH = D // P         # 8 contraction chunks of d
TT = L // P          # 16 key-token tiles
QTT = NQ // P        # 8 query-token tiles
FB = D // P          # 8 feature blocks (2 heads each)
NQC = NQ // 512      # 2 query chunks of 512
LC = L // 512        # 4 key chunks of 512
KTG = 3              # key tiles per exp batch (3 PSUM banks)

_CACHED = {}


def _bcast_rows(row_ap, n_part, n_free):
    """AP that reads a [1, n_free] DRAM row as [n_part, n_free] (stride-0)."""
    return bass.AP(tensor=row_ap.tensor, offset=row_ap.offset,
                   ap=[[0, n_part], [1, n_free]])


def _build_nc():
    nc = bacc.Bacc("TRN2", target_bir_lowering=False, debug=False, num_devices=8)

    # register eps as a float-bias constant for activation() calls
    _eps_t = nc.alloc_sbuf_tensor("const-eps", [P, 1], FP)
    nc.gpsimd.memset(_eps_t.ap(), EPS)
    nc.const_aps.aps[(FP, EPS)] = _eps_t.ap()

    xT = nc.dram_tensor("xT", [D, L], FP, kind="ExternalInput").ap()
    xskip = nc.dram_tensor("xskip", [NQ, D], FP, kind="ExternalInput").ap()
    qkvwT = nc.dram_tensor("qkvwT", [D, 3 * D], FP, kind="ExternalInput").ap()
    outwT = nc.dram_tensor("outwT", [D, D], FP, kind="ExternalInput").ap()
    normwT = nc.dram_tensor("normwT", [512, D], FP, kind="ExternalInput").ap()
    cond = nc.dram_tensor("cond", [P, 4], FP, kind="ExternalInput").ap()
    hsc2 = nc.dram_tensor("hsc2", [2, FB], FP, kind="ExternalInput").ap()
    bdiag = nc.dram_tensor("bdiag", [P, 2], BF, kind="ExternalInput").ap()
    ones1 = nc.dram_tensor("ones1", [P, 1], BF, kind="ExternalInput").ap()
    out = nc.dram_tensor("out", [NQ, D], FP, kind="ExternalOutput").ap()

    with tile.TileContext(nc) as tc:
        _body(tc, xT, xskip, qkvwT, outwT, normwT, cond, hsc2, bdiag, ones1,
              out)
    nc.compile()
    return nc


def _body(tc, xT, xskip, qkvwT, outwT, normwT, cond, hsc2, bdiag, ones1, out):
    nc = tc.nc
    mm = nc.tensor.matmul

    with (
        tc.tile_pool(name="cst", bufs=1) as cst,
        tc.tile_pool(name="pers4", bufs=1) as pers4,
        tc.tile_pool(name="drp", bufs=1, space="DRAM") as drp,
    ):
        # ---------------- constants ----------------
        hsc2_sb = cst.tile([2, FB], FP, tag="hsc2")
        nc.sync.dma_start(out=hsc2_sb[:], in_=hsc2)
        bdiag_sb = cst.tile([P, 2], BF, tag="bdiag")
        nc.sync.dma_start(out=bdiag_sb[:], in_=bdiag)
        ones_sb = cst.tile([P, 1], BF, tag="ones1")
        nc.sync.dma_start(out=ones_sb[:], in_=ones1)
        cond_sb = cst.tile([P, 4], FP, tag="cond")
        nc.sync.dma_start(out=cond_sb[:], in_=cond)
        gain_sb = cst.tile([P, DCH], FP, tag="gain")
        inv_rms = cst.tile([P, TT], FP, tag="invrms")
        sums_sb = cst.tile([P, TT], FP, tag="sums")
        denom_sb = cst.tile([NH, NQ], BF, tag="denom")

        # persistent through attention
        qT_bf = pers4.tile([P, FB, NQ], BF, tag="qT")
        kT_bf = pers4.tile([P, FB, L], BF, tag="kT")
        v_ext = pers4.tile([P, TT, NH, E + 1], BF, tag="vext")
        nc.vector.memset(v_ext[:], 1.0)  # ones col; data cells overwritten

        with tc.tile_pool(name="xtp", bufs=1) as xtp:
            xT_bf = xtp.tile([P, DCH, L], BF, tag="xT_bf")

            # ------------- stage 0/1: gain, x load+cast, inv_rms -------------
            with (
                tc.tile_pool(name="nwload", bufs=1) as nwload,
                tc.tile_pool(name="xload", bufs=2) as xload,
                tc.tile_pool(name="xsqp", bufs=1) as xsqp,
                tc.tile_pool(name="tmp1", bufs=1) as tmp1,
                tc.tile_pool(name="ps_a", bufs=2, space="PSUM") as psa,
            ):
                normw_sb = nwload.tile([P, 4, D], FP, tag="normw")
                nc.sync.dma_start(out=normw_sb[:], in_=normwT.rearrange(
                    "(c p) d -> p c d", p=P))
                for blk in range(DCH):
                    ps_g = psa.tile([P, 1], FP, tag="psg")
                    for c in range(4):
                        mm(ps_g[:], normw_sb[:, c, blk * P:(blk + 1) * P],
                           cond_sb[:, c:c + 1], start=(c == 0), stop=(c == 3))
                    nc.vector.tensor_scalar_add(gain_sb[:, blk:blk + 1],
                                                ps_g[:], 1.0)

                xsq = xsqp.tile([P, DCH, L], BF, tag="xsq")
                for dc in range(DCH):
                    xf = xload.tile([P, L], FP, tag="xf")
                    nc.sync.dma_start(out=xf[:],
                                      in_=xT[dc * P:(dc + 1) * P, :])
                    nc.vector.tensor_copy(out=xT_bf[:, dc, :], in_=xf[:])
                    nc.scalar.square(out=xsq[:, dc, :], in_=xT_bf[:, dc, :])

                for tt in range(TT):
                    ps_r = psa.tile([P, 1], FP, tag="psr")
                    for dc in range(DCH):
                        mm(ps_r[:], xsq[:, dc, tt * P:(tt + 1) * P],
                           ones_sb[:], start=(dc == 0), stop=(dc == DCH - 1))
                    nc.vector.tensor_copy(out=sums_sb[:, tt:tt + 1],
                                          in_=ps_r[:])
                # inv_rms = (sum/D + eps)^-0.5 = exp(-0.5*ln(sum/D + eps))
                lnt = tmp1.tile([P, TT], FP, tag="lnt")
                nc.scalar.activation(out=lnt[:], in_=sums_sb[:], func=AF.Ln,
                                     bias=EPS, scale=1.0 / D)
                nc.scalar.activation(out=inv_rms[:], in_=lnt[:], func=AF.Exp,
                                     scale=-0.5)

            # ------------- stage 2: qkv matmuls + per-fb cosine norms -------
            # Q and K thirds first (V last: it needs inv_rms). PSUM
            # evacuations ride on ACT (idle during this phase); the cosine
            # norms for each feature block are pipelined right behind its
            # QKV matmuls so nothing serializes at the phase boundary.
            with (
                tc.tile_pool(name="wthird", bufs=2) as wthird,
                tc.tile_pool(name="wload", bufs=2) as wload,
                tc.tile_pool(name="sqp", bufs=1) as sqp,
                tc.tile_pool(name="nstage", bufs=2) as nstage,
                tc.tile_pool(name="bcqk", bufs=1) as bcqk,
                tc.tile_pool(name="ps_qkv", bufs=4, space="PSUM") as psqkv,
                tc.tile_pool(name="ps_nrm", bufs=2, space="PSUM") as psn,
            ):
                def load_wthird(third):
                    w_bf = wthird.tile([P, DCH, D], BF, tag="wt")
                    for dc in range(DCH):
                        wf = wload.tile([P, D], FP, tag="wf")
                        nc.sync.dma_start(
                            out=wf[:],
                            in_=qkvwT[dc * P:(dc + 1) * P,
                                      third * D:(third + 1) * D])
                        nc.vector.tensor_scalar_mul(w_bf[:, dc, :], wf[:],
                                                    gain_sb[:, dc:dc + 1])
                    return w_bf

                def fb_norm(fb, t_bf, n_tok, rec_d, use_s):
                    """cosine-norm factors for the 2 heads of block fb of
                    t_bf ([P, FB, n_tok]) -> bf16 rows in DRAM rec_d, then
                    broadcast-multiply t_bf[:, fb, :] in place."""
                    sq = sqp.tile([P, n_tok], BF, tag=f"sq{n_tok}")
                    nc.vector.tensor_mul(sq[:], t_bf[:, fb, :],
                                         t_bf[:, fb, :])
                    st = nstage.tile([2, n_tok], FP, tag=f"st{n_tok}")
                    for g in range(n_tok // 1024):
                        ps_n = psn.tile([2, 1024], FP, tag="psn")
                        for c in range(2):
                            mm(ps_n[:, c * 512:(c + 1) * 512], bdiag_sb[:],
                               sq[:, (g * 2 + c) * 512:(g * 2 + c + 1) * 512],
                               start=True, stop=True)
                        nc.vector.tensor_copy(
                            out=st[:, g * 1024:(g + 1) * 1024], in_=ps_n[:])
                    # rec = s * (st + eps)^-0.5 = s * exp(-0.5*ln(st+eps))
                    nc.scalar.activation(out=st[:], in_=st[:], func=AF.Ln,
                                         bias=EPS)
                    rec = nstage.tile([2, n_tok], BF, tag=f"rec{n_tok}")
                    nc.scalar.activation(out=rec[:], in_=st[:], func=AF.Exp,
                                         scale=-0.5)
                    if use_s:
                        nc.vector.tensor_scalar_mul(rec[:], rec[:],
                                                    hsc2_sb[:, fb:fb + 1])
                    nc.sync.dma_start(out=rec_d[2 * fb:2 * fb + 2, :],
                                      in_=rec[:])
                    bc = bcqk.tile([P, n_tok], BF, tag=f"bc{n_tok}")
                    nc.sync.dma_start(
                        out=bc[0:E, :],
                        in_=_bcast_rows(rec_d[2 * fb:2 * fb + 1, :], E,
                                        n_tok))
                    nc.sync.dma_start(
                        out=bc[E:P, :],
                        in_=_bcast_rows(rec_d[2 * fb + 1:2 * fb + 2, :], E,
                                        n_tok))
                    nc.vector.tensor_mul(t_bf[:, fb, :], t_bf[:, fb, :],
                                         bc[:])

                recq_d = drp.tile([NH, NQ], BF, tag="recq_d")
                reck_d = drp.tile([NH, L], BF, tag="reck_d")

                # Q: qT[feat, q] for this core's query half (= columns 0:NQ)
                wq = load_wthird(0)
                for fb in range(FB):
                    for qc in range(NQC):
                        ps = psqkv.tile([P, 512], FP, tag="psqkv")
                        for dc in range(DCH):
                            mm(ps[:], wq[:, dc, fb * P:(fb + 1) * P],
                               xT_bf[:, dc, qc * 512:(qc + 1) * 512],
                               start=(dc == 0), stop=(dc == DCH - 1))
                        nc.scalar.copy(
                            out=qT_bf[:, fb, qc * 512:(qc + 1) * 512],
                            in_=ps[:])
                    fb_norm(fb, qT_bf, NQ, recq_d, use_s=True)

                # K: kT[feat, k] over all L tokens
                wk = load_wthird(1)
                for fb in range(FB):
                    for kc in range(LC):
                        ps = psqkv.tile([P, 512], FP, tag="psqkv")
                        for dc in range(DCH):
                            mm(ps[:], wk[:, dc, fb * P:(fb + 1) * P],
                               xT_bf[:, dc, kc * 512:(kc + 1) * 512],
                               start=(dc == 0), stop=(dc == DCH - 1))
                        nc.scalar.copy(
                            out=kT_bf[:, fb, kc * 512:(kc + 1) * 512],
                            in_=ps[:])
                    fb_norm(fb, kT_bf, L, reck_d, use_s=False)

                # V: v[tok, feat] natural, scaled by inv_rms, into v_ext
                wv = load_wthird(2)
                for tt in range(TT):
                    for vc in range(2):
                        ps = psqkv.tile([P, 512], FP, tag="psqkv")
                        for dc in range(DCH):
                            mm(ps[:], xT_bf[:, dc, tt * P:(tt + 1) * P],
                               wv[:, dc, vc * 512:(vc + 1) * 512],
                               start=(dc == 0), stop=(dc == DCH - 1))
                        nc.vector.tensor_scalar_mul(
                            v_ext[:, tt, vc * 8:(vc + 1) * 8, 0:E],
                            ps[:].rearrange("p (h e) -> p h e", e=E),
                            inv_rms[:, tt:tt + 1])

        # ---------------- stage 4: attention ----------------
        with (
            tc.tile_pool(name="o4p", bufs=1) as o4p,
            tc.tile_pool(name="owload", bufs=2) as owload,
            tc.tile_pool(name="attnp", bufs=2) as attnp,
            tc.tile_pool(name="ottp", bufs=3) as ottp,
            tc.tile_pool(name="bcdp", bufs=2) as bcdp,
        ):
            outw_bf = o4p.tile([P, FB, D], BF, tag="outw")
            for dc in range(DCH):
                owf = owload.tile([P, D], FP, tag="owf")
                nc.sync.dma_start(out=owf[:],
                                  in_=outwT[dc * P:(dc + 1) * P, :])
                nc.vector.tensor_copy(out=outw_bf[:, dc, :], in_=owf[:])
            oT_sb = o4p.tile([P, FB, NQ], BF, tag="oT")

            groups = [list(range(g * KTG, min(TT, (g + 1) * KTG)))
                      for g in range((TT + KTG - 1) // KTG)]
            NG = len(groups)
            with (
                tc.tile_pool(name="ps_sc", bufs=2, space="PSUM") as pssc,
                tc.tile_pool(name="ps_ot", bufs=1, space="PSUM") as psot,
                tc.tile_pool(name="ps_dm", bufs=1, space="PSUM") as psdm,
            ):
                # Scratch bank for filler matmuls. The attention phase is
                # ACT(exp)-bound; without filler the PE takes a short exp-wait
                # every pipeline round, HAM never sees a full busy window, and
                # the PE gets stuck at the 1.2 GHz cold clock. The fillers
                # absorb exactly the ACT-PE rate difference and keep the
                # activity monitor hot. Nothing reads their output.
                dmy = psdm.tile([2, 512], FP, tag="dmy")

                def emit_dummy():
                    mm(dmy[:], bdiag_sb[:], kT_bf[:, 0, 0:512],
                       start=True, stop=True)

                for h in range(NH):
                    fb, hh = h // 2, h % 2
                    pb = E * hh
                    for qc in range(NQC):
                        att = attnp.tile([P, TT * 512], BF, tag="att")
                        pso = psot.tile([E + 1, 512], FP, tag="psot")

                        def emit_scores(g):
                            kts = groups[g]
                            ps = pssc.tile([P, KTG * 512], FP, tag="pssc")
                            for j, kt in enumerate(kts):
                                mm(ps[:, j * 512:(j + 1) * 512],
                                   kT_bf[pb:pb + E, fb, kt * P:(kt + 1) * P],
                                   qT_bf[pb:pb + E, fb,
                                         qc * 512:(qc + 1) * 512],
                                   start=True, stop=True)
                            nc.scalar.activation(
                                out=att[:, kts[0] * 512:
                                        (kts[-1] + 1) * 512],
                                in_=ps[:, :len(kts) * 512], func=AF.Exp)

                        def emit_av(g):
                            for kt in groups[g]:
                                mm(pso[:], v_ext[:, kt, h, :],
                                   att[:, kt * 512:(kt + 1) * 512],
                                   start=(kt == 0), stop=(kt == TT - 1))

                        # software pipeline: scores(g+2) + filler + av(g)
                        emit_scores(0)
                        emit_scores(1)
                        for g in range(NG):
                            if g + 2 < NG:
                                emit_scores(g + 2)
                            emit_dummy()
                            if g >= NG - 2:
                                emit_dummy()
                            emit_av(g)

                        ot = ottp.tile([E + 1, 512], BF, tag="ottmp")
                        nc.vector.tensor_copy(out=ot[:], in_=pso[:])
                        nc.sync.dma_start(
                            out=oT_sb[pb:pb + E, fb, qc * 512:(qc + 1) * 512],
                            in_=ot[0:E, :])
                        nc.sync.dma_start(
                            out=denom_sb[h:h + 1, qc * 512:(qc + 1) * 512],
                            in_=ot[E:E + 1, :])

            # normalize oT by 1/denom
            ld = ottp.tile([NH, NQ], FP, tag="ld")
            nc.scalar.activation(out=ld[:], in_=denom_sb[:], func=AF.Ln)
            recd_bf = ottp.tile([NH, NQ], BF, tag="recd")
            nc.scalar.activation(out=recd_bf[:], in_=ld[:], func=AF.Exp,
                                 scale=-1.0)
            recd_d = drp.tile([NH, NQ], BF, tag="recd_d")
            nc.sync.dma_start(out=recd_d[:], in_=recd_bf[:])
            for h in range(NH):
                fb, hh = h // 2, h % 2
                pb = E * hh
                bcd = bcdp.tile([P, NQ], BF, tag="bcd")
                nc.sync.dma_start(
                    out=bcd[pb:pb + E, :],
                    in_=_bcast_rows(recd_d[h:h + 1, :], E, NQ))
                nc.vector.tensor_mul(oT_sb[pb:pb + E, fb, :],
                                     oT_sb[pb:pb + E, fb, :],
                                     bcd[pb:pb + E, :])

            # ------------- stage 5: out projection + residual -------------
            with (
                tc.tile_pool(name="skipp", bufs=2) as skipp,
                tc.tile_pool(name="outp", bufs=2) as outp,
                tc.tile_pool(name="ps_out", bufs=4, space="PSUM") as psout,
            ):
                for tq in range(QTT):
                    xs = skipp.tile([P, D], FP, tag="xs")
                    nc.sync.dma_start(out=xs[:],
                                      in_=xskip[tq * P:(tq + 1) * P, :])
                    osb = outp.tile([P, D], FP, tag="osb")
                    for dc2 in range(2):
                        ps = psout.tile([P, 512], FP, tag="psout")
                        for fb in range(FB):
                            mm(ps[:], oT_sb[:, fb, tq * P:(tq + 1) * P],
                               outw_bf[:, fb, dc2 * 512:(dc2 + 1) * 512],
                               start=(fb == 0), stop=(fb == FB - 1))
                        nc.vector.tensor_add(
                            osb[:, dc2 * 512:(dc2 + 1) * 512], ps[:],
                            xs[:, dc2 * 512:(dc2 + 1) * 512])
                    nc.sync.dma_start(out=out[tq * P:(tq + 1) * P, :],
                                      in_=osb[:])


def _make_in_maps(x, cond_norm, norm_w, qkv_w, head_scale, out_w):
    qkvwT = np.ascontiguousarray(qkv_w.T)
    outwT = np.ascontiguousarray(out_w.T)
    normwT = np.ascontiguousarray(norm_w.T)
    # hsc2[hh, fb] = head_scale[2*fb + hh]
    hsc2 = np.ascontiguousarray(head_scale.reshape(FB, 2).T)
    bd = np.zeros((P, 2), dtype=ml_dtypes.bfloat16)
    bd[0:E, 0] = 1.0
    bd[E:P, 1] = 1.0
    ones1 = np.ones((P, 1), dtype=ml_dtypes.bfloat16)

    in_maps = []
    for core in range(8):
        b, half = core // 2, core % 2
        xTb = x[b].T  # [D, L]
        if half == 0:
            xTr = np.ascontiguousarray(xTb)
        else:
            # rotate so this core's query half occupies columns 0..NQ-1
            xTr = np.ascontiguousarray(
                np.concatenate([xTb[:, NQ:], xTb[:, :NQ]], axis=1))
        in_maps.append({
            "xT": xTr,
            "xskip": np.ascontiguousarray(x[b, half * NQ:(half + 1) * NQ, :]),
            "qkvwT": qkvwT,
            "outwT": outwT,
            "normwT": normwT,
            "cond": np.ascontiguousarray(cond_norm[b].reshape(4, P).T),
            "hsc2": hsc2,
            "bdiag": bd,
            "ones1": ones1,
        })
    return in_maps


def get_nc():
    if "nc" not in _CACHED:
        _CACHED["nc"] = _build_nc()
    return _CACHED["nc"]


def run(inputs, trace=False):
    """Returns (full_output, BassKernelResults)."""
    x = np.asarray(inputs["x"], dtype=np.float32)
    in_maps = _make_in_maps(
        x,
        np.asarray(inputs["cond_norm"], dtype=np.float32),
        np.asarray(inputs["norm_w"], dtype=np.float32),
        np.asarray(inputs["qkv_w"], dtype=np.float32),
        np.asarray(inputs["head_scale"], dtype=np.float32),
        np.asarray(inputs["out_w"], dtype=np.float32),
    )
    nc = get_nc()
    res = run_bass_kernel_spmd(nc, in_maps, core_ids=list(range(8)),
                               trace=trace)
    full = np.empty((N_B, L, D), dtype=np.float32)
    for core in range(8):
        b, half = core // 2, core % 2
        full[b, half * NQ:(half + 1) * NQ, :] = res.results[core]["out"]
    return full, res


def kernel(**inputs) -> np.ndarray:
    full, _ = run(inputs, trace=False)
    return full


# revision 16
# speedup vs baseline: 1.2024x; 1.2024x over previous
"""Trainium2 Bass kernel for nn_AttentionBlock_29291676959393.

Computation (per batch b):
  gain = cond_norm[b] @ norm_w.T + 1            [D]
  xn   = x * gain * rsqrt(mean(x^2, -1) + eps)
  q,k,v = split(xn @ qkv_w.T)                   heads of 64
  q,k  = cosine-normalized * sqrt(head_scale)
  out  = softmax(q @ k.T) @ v @ out_w.T + x

Sharding: 8 cores = (batch 0..3) x (query-token half 0..1). Each core runs
all 16 heads for its 1024 query tokens; K/V are computed redundantly by the
two cores sharing a batch (no collectives at all).

Device-side layout choices:
  - Host passes transposed weight/activation views (pure np layout work), so
    the device never transposes anything. The host also rotates the token
    axis of x.T per-core so the core's own query half occupies columns
    0..1023 (attention is invariant to key-token permutation as long as K
    and V share it), letting one NEFF serve both halves.
  - gain folds into the qkv weight (per-partition scale in [d, feat] layout).
  - inv_rms folds into V only (cosine norm makes Q/K invariant to it).
  - scores are computed transposed [k, q]; the softmax denominator rides the
    attn@v matmul as a 65th ones-column of V (M=65); no max-subtraction is
    needed since cosine-sim scores are bounded by head_scale=10.
  - rsqrt/recip computed as exp(-a*ln(x)) on ACT (one table set with Exp).
  - all matmuls bf16 with f32 PSUM accumulation.
  - attention inner loop is software-pipelined at kt-group granularity
    (scores g+1 interleaved with attn@v g) so the in-order PE never sits in
    multi-us exp waits (keeps the HAM clock at 2.4 GHz).
"""

import numpy as np
import ml_dtypes

import concourse.bass as bass
import concourse.bacc as bacc
import concourse.tile as tile
from concourse import mybir
from concourse.bass_utils import run_bass_kernel_spmd

FP = mybir.dt.float32
BF = mybir.dt.bfloat16
AF = mybir.ActivationFunctionType

P = 128
N_B, L, D = 4, 2048, 1024
NH, E = 16, 64
NQ = L // 2          # query tokens per core
EPS = 1e-6
DCH = D // P         # 8 contraction chunks of d
TT = L // P          # 16 key-token tiles
QTT = NQ // P        # 8 query-token tiles
FB = D // P          # 8 feature blocks (2 heads each)
NQC = NQ // 512      # 2 query chunks of 512
LC = L // 512        # 4 key chunks of 512
KTG = 3              # key tiles per exp batch (3 PSUM banks)

_CACHED = {}


def _bcast_rows(row_ap, n_part, n_free):
    """AP that reads a [1, n_free] DRAM row as [n_part, n_free] (stride-0)."""
    return bass.AP(tensor=row_ap.tensor, offset=row_ap.offset,
                   ap=[[0, n_part], [1, n_free]])


def _build_nc():
    nc = bacc.Bacc("TRN2", target_bir_lowering=False, debug=False, num_devices=8)

    # register eps as a float-bias constant for activation() calls
    _eps_t = nc.alloc_sbuf_tensor("const-eps", [P, 1], FP)
    nc.gpsimd.memset(_eps_t.ap(), EPS)
    nc.const_aps.aps[(FP, EPS)] = _eps_t.ap()

    xT = nc.dram_tensor("xT", [D, L], FP, kind="ExternalInput").ap()
    xskip = nc.dram_tensor("xskip", [NQ, D], FP, kind="ExternalInput").ap()
    qkvwT = nc.dram_tensor("qkvwT", [D, 3 * D], FP, kind="ExternalInput").ap()
    outwT = nc.dram_tensor("outwT", [D, D], FP, kind="ExternalInput").ap()
    normwT = nc.dram_tensor("normwT", [512, D], FP, kind="ExternalInput").ap()
    cond = nc.dram_tensor("cond", [P, 4], FP, kind="ExternalInput").ap()
    hsc2 = nc.dram_tensor("hsc2", [2, FB], FP, kind="ExternalInput").ap()
    bdiag = nc.dram_tensor("bdiag", [P, 2], BF, kind="ExternalInput").ap()
    ones1 = nc.dram_tensor("ones1", [P, 1], BF, kind="ExternalInput").ap()
    out = nc.dram_tensor("out", [NQ, D], FP, kind="ExternalOutput").ap()

    with tile.TileContext(nc) as tc:
        _body(tc, xT, xskip, qkvwT, outwT, normwT, cond, hsc2, bdiag, ones1,
              out)
    nc.compile()
    return nc


def _body(tc, xT, xskip, qkvwT, outwT, normwT, cond, hsc2, bdiag, ones1, out):
    nc = tc.nc
    mm = nc.tensor.matmul

    with (
        tc.tile_pool(name="cst", bufs=1) as cst,
        tc.tile_pool(name="pers4", bufs=1) as pers4,
        tc.tile_pool(name="drp", bufs=1, space="DRAM") as drp,
    ):
        # ---------------- constants ----------------
        hsc2_sb = cst.tile([2, FB], FP, tag="hsc2")
        nc.sync.dma_start(out=hsc2_sb[:], in_=hsc2)
        bdiag_sb = cst.tile([P, 2], BF, tag="bdiag")
        nc.sync.dma_start(out=bdiag_sb[:], in_=bdiag)
        ones_sb = cst.tile([P, 1], BF, tag="ones1")
        nc.sync.dma_start(out=ones_sb[:], in_=ones1)
        cond_sb = cst.tile([P, 4], FP, tag="cond")
        nc.sync.dma_start(out=cond_sb[:], in_=cond)
        gain_sb = cst.tile([P, DCH], FP, tag="gain")
        inv_rms = cst.tile([P, TT], FP, tag="invrms")
        sums_sb = cst.tile([P, TT], FP, tag="sums")
        denom_sb = cst.tile([NH, NQ], BF, tag="denom")

        # persistent through attention
        qT_bf = pers4.tile([P, FB, NQ], BF, tag="qT")
        kT_bf = pers4.tile([P, FB, L], BF, tag="kT")
        v_ext = pers4.tile([P, TT, NH, E + 1], BF, tag="vext")
        nc.vector.memset(v_ext[:], 1.0)  # ones col; data cells overwritten

        with tc.tile_pool(name="xtp", bufs=1) as xtp:
            xT_bf = xtp.tile([P, DCH, L], BF, tag="xT_bf")

            # ------------- stage 0/1: gain, x load+cast, inv_rms -------------
            with (
                tc.tile_pool(name="nwload", bufs=1) as nwload,
                tc.tile_pool(name="xload", bufs=2) as xload,
                tc.tile_pool(name="xsqp", bufs=1) as xsqp,
                tc.tile_pool(name="tmp1", bufs=1) as tmp1,
                tc.tile_pool(name="ps_a", bufs=2, space="PSUM") as psa,
            ):
                normw_sb = nwload.tile([P, 4, D], FP, tag="normw")
                nc.sync.dma_start(out=normw_sb[:], in_=normwT.rearrange(
                    "(c p) d -> p c d", p=P))
                for blk in range(DCH):
                    ps_g = psa.tile([P, 1], FP, tag="psg")
                    for c in range(4):
                        mm(ps_g[:], normw_sb[:, c, blk * P:(blk + 1) * P],
                           cond_sb[:, c:c + 1], start=(c == 0), stop=(c == 3))
                    nc.vector.tensor_scalar_add(gain_sb[:, blk:blk + 1],
                                                ps_g[:], 1.0)

                xsq = xsqp.tile([P, DCH, L], BF, tag="xsq")
                for dc in range(DCH):
                    xf = xload.tile([P, L], FP, tag="xf")
                    nc.sync.dma_start(out=xf[:],
                                      in_=xT[dc * P:(dc + 1) * P, :])
                    nc.vector.tensor_copy(out=xT_bf[:, dc, :], in_=xf[:])
                    nc.scalar.square(out=xsq[:, dc, :], in_=xT_bf[:, dc, :])

                for tt in range(TT):
                    ps_r = psa.tile([P, 1], FP, tag="psr")
                    for dc in range(DCH):
                        mm(ps_r[:], xsq[:, dc, tt * P:(tt + 1) * P],
                           ones_sb[:], start=(dc == 0), stop=(dc == DCH - 1))
                    nc.vector.tensor_copy(out=sums_sb[:, tt:tt + 1],
                                          in_=ps_r[:])
                # inv_rms = (sum/D + eps)^-0.5 = exp(-0.5*ln(sum/D + eps))
                lnt = tmp1.tile([P, TT], FP, tag="lnt")
                nc.scalar.activation(out=lnt[:], in_=sums_sb[:], func=AF.Ln,
                                     bias=EPS, scale=1.0 / D)
                nc.scalar.activation(out=inv_rms[:], in_=lnt[:], func=AF.Exp,
                                     scale=-0.5)

            # ------------- stage 2: qkv matmuls + per-fb cosine norms -------
            # Q and K thirds first (V last: it needs inv_rms). PSUM
            # evacuations ride on ACT (idle during this phase); the cosine
            # norms for each feature block are pipelined right behind its
            # QKV matmuls so nothing serializes at the phase boundary.
            with (
                tc.tile_pool(name="wthird", bufs=2) as wthird,
                tc.tile_pool(name="wload", bufs=2) as wload,
                tc.tile_pool(name="sqp", bufs=2) as sqp,
                tc.tile_pool(name="nstage", bufs=2) as nstage,
                tc.tile_pool(name="bcqk", bufs=1) as bcqk,
                tc.tile_pool(name="ps_qkv", bufs=4, space="PSUM") as psqkv,
                tc.tile_pool(name="ps_nrm", bufs=2, space="PSUM") as psn,
            ):
                def load_wthird(third):
                    w_bf = wthird.tile([P, DCH, D], BF, tag="wt")
                    for dc in range(DCH):
                        wf = wload.tile([P, D], FP, tag="wf")
                        nc.sync.dma_start(
                            out=wf[:],
                            in_=qkvwT[dc * P:(dc + 1) * P,
                                      third * D:(third + 1) * D])
                        nc.vector.tensor_scalar_mul(w_bf[:, dc, :], wf[:],
                                                    gain_sb[:, dc:dc + 1])
                    return w_bf

                def fb_norm(fb, t_bf, n_tok, rec_d, use_s):
                    """cosine-norm factors for the 2 heads of block fb of
                    t_bf ([P, FB, n_tok]) -> bf16 rows in DRAM rec_d, then
                    broadcast-multiply t_bf[:, fb, :] in place."""
                    sq = sqp.tile([P, n_tok], BF, tag=f"sq{n_tok}")
                    nc.vector.tensor_mul(sq[:], t_bf[:, fb, :],
                                         t_bf[:, fb, :])
                    st = nstage.tile([2, n_tok], BF, tag=f"st{n_tok}")
                    for g in range(n_tok // 1024):
                        ps_n = psn.tile([2, 1024], FP, tag="psn")
                        for c in range(2):
                            mm(ps_n[:, c * 512:(c + 1) * 512], bdiag_sb[:],
                               sq[:, (g * 2 + c) * 512:(g * 2 + c + 1) * 512],
                               start=True, stop=True)
                        nc.vector.tensor_copy(
                            out=st[:, g * 1024:(g + 1) * 1024], in_=ps_n[:])
                    # rec = s * (st + eps)^-0.5 = s * exp(-0.5*ln(st+eps))
                    nc.scalar.activation(out=st[:], in_=st[:], func=AF.Ln,
                                         bias=EPS)
                    rec = nstage.tile([2, n_tok], BF, tag=f"rec{n_tok}")
                    nc.scalar.activation(out=rec[:], in_=st[:], func=AF.Exp,
                                         scale=-0.5)
                    if use_s:
                        nc.vector.tensor_scalar_mul(rec[:], rec[:],
                                                    hsc2_sb[:, fb:fb + 1])
                    nc.sync.dma_start(out=rec_d[2 * fb:2 * fb + 2, :],
                                      in_=rec[:])
                    bc = bcqk.tile([P, n_tok], BF, tag=f"bc{n_tok}")
                    nc.sync.dma_start(
                        out=bc[0:E, :],
                        in_=_bcast_rows(rec_d[2 * fb:2 * fb + 1, :], E,
                                        n_tok))
                    nc.sync.dma_start(
                        out=bc[E:P, :],
                        in_=_bcast_rows(rec_d[2 * fb + 1:2 * fb + 2, :], E,
                                        n_tok))
                    nc.vector.tensor_mul(t_bf[:, fb, :], t_bf[:, fb, :],
                                         bc[:])

                recq_d = drp.tile([NH, NQ], BF, tag="recq_d")
                reck_d = drp.tile([NH, L], BF, tag="reck_d")

                # Q: qT[feat, q] for this core's query half (= columns 0:NQ)
                wq = load_wthird(0)
                for fb in range(FB):
                    for qc in range(NQC):
                        ps = psqkv.tile([P, 512], FP, tag="psqkv")
                        for dc in range(DCH):
                            mm(ps[:], wq[:, dc, fb * P:(fb + 1) * P],
                               xT_bf[:, dc, qc * 512:(qc + 1) * 512],
                               start=(dc == 0), stop=(dc == DCH - 1))
                        nc.vector.tensor_copy(
                            out=qT_bf[:, fb, qc * 512:(qc + 1) * 512],
                            in_=ps[:])
                    fb_norm(fb, qT_bf, NQ, recq_d, use_s=True)

                # K: kT[feat, k] over all L tokens
                wk = load_wthird(1)
                for fb in range(FB):
                    for kc in range(LC):
                        ps = psqkv.tile([P, 512], FP, tag="psqkv")
                        for dc in range(DCH):
                            mm(ps[:], wk[:, dc, fb * P:(fb + 1) * P],
                               xT_bf[:, dc, kc * 512:(kc + 1) * 512],
                               start=(dc == 0), stop=(dc == DCH - 1))
                        nc.vector.tensor_copy(
                            out=kT_bf[:, fb, kc * 512:(kc + 1) * 512],
                            in_=ps[:])
                    fb_norm(fb, kT_bf, L, reck_d, use_s=False)

                # V: v[tok, feat] natural, scaled by inv_rms, into v_ext
                wv = load_wthird(2)
                for tt in range(TT):
                    for vc in range(2):
                        ps = psqkv.tile([P, 512], FP, tag="psqkv")
                        for dc in range(DCH):
                            mm(ps[:], xT_bf[:, dc, tt * P:(tt + 1) * P],
                               wv[:, dc, vc * 512:(vc + 1) * 512],
                               start=(dc == 0), stop=(dc == DCH - 1))
                        nc.vector.tensor_scalar_mul(
                            v_ext[:, tt, vc * 8:(vc + 1) * 8, 0:E],
                            ps[:].rearrange("p (h e) -> p h e", e=E),
                            inv_rms[:, tt:tt + 1])

        # ---------------- stage 4: attention ----------------
        with (
            tc.tile_pool(name="o4p", bufs=1) as o4p,
            tc.tile_pool(name="owload", bufs=2) as owload,
            tc.tile_pool(name="attnp", bufs=4) as attnp,
            tc.tile_pool(name="ottp", bufs=3) as ottp,
            tc.tile_pool(name="bcdp", bufs=2) as bcdp,
        ):
            outw_bf = o4p.tile([P, FB, D], BF, tag="outw")
            for dc in range(DCH):
                owf = owload.tile([P, D], FP, tag="owf")
                nc.sync.dma_start(out=owf[:],
                                  in_=outwT[dc * P:(dc + 1) * P, :])
                nc.vector.tensor_copy(out=outw_bf[:, dc, :], in_=owf[:])
            oT_sb = o4p.tile([P, FB, NQ], BF, tag="oT")

            groups = [list(range(g * KTG, min(TT, (g + 1) * KTG)))
                      for g in range((TT + KTG - 1) // KTG)]
            NG = len(groups)
            with (
                tc.tile_pool(name="ps_sc", bufs=2, space="PSUM") as pssc,
                tc.tile_pool(name="ps_ot", bufs=1, space="PSUM") as psot,
                tc.tile_pool(name="ps_dm", bufs=1, space="PSUM") as psdm,
            ):
                # Scratch bank for filler matmuls. The attention phase is
                # ACT(exp)-bound; without filler the PE takes a short exp-wait
                # every pipeline round, HAM never sees a full busy window, and
                # the PE drops to the 1.2 GHz cold clock. The fillers absorb
                # the ACT-PE rate difference; nothing reads their output.
                dmy = psdm.tile([2, 512], FP, tag="dmy")

                def emit_dummy():
                    mm(dmy[:], bdiag_sb[:], kT_bf[:, 0, 0:512],
                       start=True, stop=True)

                # Global software pipeline over (head, q-chunk, kt-group)
                # tasks: scores+exp run LAG task-slots ahead of attn@v, so
                # neither engine stalls at iteration boundaries.
                state = {}

                def sc_task(h, qc, g):
                    fb, hh = h // 2, h % 2
                    pb = E * hh
                    kts = groups[g]
                    ps = pssc.tile([P, KTG * 512], FP, tag="pssc")
                    for j, kt in enumerate(kts):
                        mm(ps[:, j * 512:(j + 1) * 512],
                           kT_bf[pb:pb + E, fb, kt * P:(kt + 1) * P],
                           qT_bf[pb:pb + E, fb, qc * 512:(qc + 1) * 512],
                           start=True, stop=True)
                    attg = attnp.tile([P, KTG * 512], BF, tag="attg",
                                      name=f"attg_{h}_{qc}_{g}")
                    state[(h, qc, g)] = attg
                    nc.scalar.activation(out=attg[:, :len(kts) * 512],
                                         in_=ps[:, :len(kts) * 512],
                                         func=AF.Exp)

                def av_task(h, qc, g):
                    fb, hh = h // 2, h % 2
                    pb = E * hh
                    if g == 0:
                        state[(h, qc)] = psot.tile(
                            [E + 1, 512], FP, tag="psot",
                            name=f"psot_{h}_{qc}")
                    pso = state[(h, qc)]
                    attg = state.pop((h, qc, g))
                    for j, kt in enumerate(groups[g]):
                        mm(pso[:], v_ext[:, kt, h, :],
                           attg[:, j * 512:(j + 1) * 512],
                           start=(kt == 0), stop=(kt == TT - 1))
                    if g == NG - 1:
                        pso = state.pop((h, qc))
                        ot = ottp.tile([E + 1, 512], BF, tag="ottmp")
                        nc.vector.tensor_copy(out=ot[:], in_=pso[:])
                        nc.sync.dma_start(
                            out=oT_sb[pb:pb + E, fb, qc * 512:(qc + 1) * 512],
                            in_=ot[0:E, :])
                        nc.sync.dma_start(
                            out=denom_sb[h:h + 1, qc * 512:(qc + 1) * 512],
                            in_=ot[E:E + 1, :])

                flat = [(h, qc, g) for h in range(NH) for qc in range(NQC)
                        for g in range(NG)]
                LAG = 2
                for idx in range(len(flat) + LAG):
                    if idx < len(flat):
                        sc_task(*flat[idx])
                    if idx >= LAG:
                        h2, qc2, g2 = flat[idx - LAG]
                        emit_dummy()
                        if g2 % 2 == 0:
                            emit_dummy()
                        av_task(h2, qc2, g2)

            # normalize oT by 1/denom
            ld = ottp.tile([NH, NQ], FP, tag="ld")
            nc.scalar.activation(out=ld[:], in_=denom_sb[:], func=AF.Ln)
            recd_bf = ottp.tile([NH, NQ], BF, tag="recd")
            nc.scalar.activation(out=recd_bf[:], in_=ld[:], func=AF.Exp,
                                 scale=-1.0)
            recd_d = drp.tile([NH, NQ], BF, tag="recd_d")
            nc.sync.dma_start(out=recd_d[:], in_=recd_bf[:])
            for h in range(NH):
                fb, hh = h // 2, h % 2
                pb = E * hh
                bcd = bcdp.tile([P, NQ], BF, tag="bcd")
                nc.sync.dma_start(
                    out=bcd[pb:pb + E, :],
                    in_=_bcast_rows(recd_d[h:h + 1, :], E, NQ))
                nc.vector.tensor_mul(oT_sb[pb:pb + E, fb, :],
                                     oT_sb[pb:pb + E, fb, :],
                                     bcd[pb:pb + E, :])

            # ------------- stage 5: out projection + residual -------------
            with (
                tc.tile_pool(name="skipp", bufs=2) as skipp,
                tc.tile_pool(name="outp", bufs=2) as outp,
                tc.tile_pool(name="ps_out", bufs=4, space="PSUM") as psout,
            ):
                for tq in range(QTT):
                    xs = skipp.tile([P, D], FP, tag="xs")
                    nc.sync.dma_start(out=xs[:],
                                      in_=xskip[tq * P:(tq + 1) * P, :])
                    osb = outp.tile([P, D], FP, tag="osb")
                    for dc2 in range(2):
                        ps = psout.tile([P, 512], FP, tag="psout")
                        for fb in range(FB):
                            mm(ps[:], oT_sb[:, fb, tq * P:(tq + 1) * P],
                               outw_bf[:, fb, dc2 * 512:(dc2 + 1) * 512],
                               start=(fb == 0), stop=(fb == FB - 1))
                        nc.vector.tensor_add(
                            osb[:, dc2 * 512:(dc2 + 1) * 512], ps[:],
                            xs[:, dc2 * 512:(dc2 + 1) * 512])
                    nc.sync.dma_start(out=out[tq * P:(tq + 1) * P, :],
                                      in_=osb[:])


def _make_in_maps(x, cond_norm, norm_w, qkv_w, head_scale, out_w):
    qkvwT = np.ascontiguousarray(qkv_w.T)
    outwT = np.ascontiguousarray(out_w.T)
    normwT = np.ascontiguousarray(norm_w.T)
    # hsc2[hh, fb] = head_scale[2*fb + hh]
    hsc2 = np.ascontiguousarray(head_scale.reshape(FB, 2).T)
    bd = np.zeros((P, 2), dtype=ml_dtypes.bfloat16)
    bd[0:E, 0] = 1.0
    bd[E:P, 1] = 1.0
    ones1 = np.ones((P, 1), dtype=ml_dtypes.bfloat16)

    in_maps = []
    for core in range(8):
        b, half = core // 2, core % 2
        xTb = x[b].T  # [D, L]
        if half == 0:
            xTr = np.ascontiguousarray(xTb)
        else:
            # rotate so this core's query half occupies columns 0..NQ-1
            xTr = np.ascontiguousarray(
                np.concatenate([xTb[:, NQ:], xTb[:, :NQ]], axis=1))
        in_maps.append({
            "xT": xTr,
            "xskip": np.ascontiguousarray(x[b, half * NQ:(half + 1) * NQ, :]),
            "qkvwT": qkvwT,
            "outwT": outwT,
            "normwT": normwT,
            "cond": np.ascontiguousarray(cond_norm[b].reshape(4, P).T),
            "hsc2": hsc2,
            "bdiag": bd,
            "ones1": ones1,
        })
    return in_maps


def get_nc():
    if "nc" not in _CACHED:
        _CACHED["nc"] = _build_nc()
    return _CACHED["nc"]


def run(inputs, trace=False):
    """Returns (full_output, BassKernelResults)."""
    x = np.asarray(inputs["x"], dtype=np.float32)
    in_maps = _make_in_maps(
        x,
        np.asarray(inputs["cond_norm"], dtype=np.float32),
        np.asarray(inputs["norm_w"], dtype=np.float32),
        np.asarray(inputs["qkv_w"], dtype=np.float32),
        np.asarray(inputs["head_scale"], dtype=np.float32),
        np.asarray(inputs["out_w"], dtype=np.float32),
    )
    nc = get_nc()
    res = run_bass_kernel_spmd(nc, in_maps, core_ids=list(range(8)),
                               trace=trace)
    full = np.empty((N_B, L, D), dtype=np.float32)
    for core in range(8):
        b, half = core // 2, core % 2
        full[b, half * NQ:(half + 1) * NQ, :] = res.results[core]["out"]
    return full, res


def kernel(**inputs) -> np.ndarray:
    full, _ = run(inputs, trace=False)
    return full


# revision 17
# speedup vs baseline: 1.3446x; 1.1183x over previous
"""Trainium2 Bass kernel for nn_AttentionBlock_29291676959393.

Computation (per batch b):
  gain = cond_norm[b] @ norm_w.T + 1            [D]
  xn   = x * gain * rsqrt(mean(x^2, -1) + eps)
  q,k,v = split(xn @ qkv_w.T)                   heads of 64
  q,k  = cosine-normalized * sqrt(head_scale)
  out  = softmax(q @ k.T) @ v @ out_w.T + x

Sharding: 8 cores = (batch 0..3) x (query-token half 0..1). Each core runs
all 16 heads for its 1024 query tokens; K/V are computed redundantly by the
two cores sharing a batch (no collectives at all).

Device-side layout choices:
  - Host passes transposed weight/activation views (pure np layout work), so
    the device never transposes anything. The host also rotates the token
    axis of x.T per-core so the core's own query half occupies columns
    0..1023 (attention is invariant to key-token permutation as long as K
    and V share it), letting one NEFF serve both halves.
  - gain folds into the qkv weight (per-partition scale in [d, feat] layout).
  - inv_rms folds into V only (cosine norm makes Q/K invariant to it).
  - scores are computed transposed [k, q]; the softmax denominator rides the
    attn@v matmul as a 65th ones-column of V (M=65); no max-subtraction is
    needed since cosine-sim scores are bounded by head_scale=10.
  - rsqrt/recip computed as exp(-a*ln(x)) on ACT (one table set with Exp).
  - all matmuls bf16 with f32 PSUM accumulation.
  - attention inner loop is software-pipelined at kt-group granularity
    (scores g+1 interleaved with attn@v g) so the in-order PE never sits in
    multi-us exp waits (keeps the HAM clock at 2.4 GHz).
"""

import numpy as np
import ml_dtypes

import concourse.bass as bass
import concourse.bacc as bacc
import concourse.tile as tile
from concourse import mybir
from concourse.bass_utils import run_bass_kernel_spmd

FP = mybir.dt.float32
BF = mybir.dt.bfloat16
AF = mybir.ActivationFunctionType

P = 128
N_B, L, D = 4, 2048, 1024
NH, E = 16, 64
NQ = L // 2          # query tokens per core
EPS = 1e-6
DCH = D // P         # 8 contraction chunks of d
TT = L // P          # 16 key-token tiles
QTT = NQ // P        # 8 query-token tiles
FB = D // P          # 8 feature blocks (2 heads each)
NQC = NQ // 512      # 2 query chunks of 512
LC = L // 512        # 4 key chunks of 512
KTG = 3              # key tiles per exp batch (3 PSUM banks)

_CACHED = {}


def _bcast_rows(row_ap, n_part, n_free):
    """AP that reads a [1, n_free] DRAM row as [n_part, n_free] (stride-0)."""
    return bass.AP(tensor=row_ap.tensor, offset=row_ap.offset,
                   ap=[[0, n_part], [1, n_free]])


def _build_nc():
    nc = bacc.Bacc("TRN2", target_bir_lowering=False, debug=False, num_devices=8)

    # register eps as a float-bias constant for activation() calls
    _eps_t = nc.alloc_sbuf_tensor("const-eps", [P, 1], FP)
    nc.gpsimd.memset(_eps_t.ap(), EPS)
    nc.const_aps.aps[(FP, EPS)] = _eps_t.ap()

    xT = nc.dram_tensor("xT", [D, L], FP, kind="ExternalInput").ap()
    xskip = nc.dram_tensor("xskip", [NQ, D], FP, kind="ExternalInput").ap()
    qkvwT = nc.dram_tensor("qkvwT", [D, 3 * D], FP, kind="ExternalInput").ap()
    outwT = nc.dram_tensor("outwT", [D, D], FP, kind="ExternalInput").ap()
    normwT = nc.dram_tensor("normwT", [512, D], FP, kind="ExternalInput").ap()
    cond = nc.dram_tensor("cond", [P, 4], FP, kind="ExternalInput").ap()
    hsc2 = nc.dram_tensor("hsc2", [2, FB], FP, kind="ExternalInput").ap()
    bdiag = nc.dram_tensor("bdiag", [P, 2], BF, kind="ExternalInput").ap()
    ones1 = nc.dram_tensor("ones1", [P, 1], BF, kind="ExternalInput").ap()
    out = nc.dram_tensor("out", [NQ, D], FP, kind="ExternalOutput").ap()

    with tile.TileContext(nc) as tc:
        _body(tc, xT, xskip, qkvwT, outwT, normwT, cond, hsc2, bdiag, ones1,
              out)
    nc.compile()
    return nc


def _body(tc, xT, xskip, qkvwT, outwT, normwT, cond, hsc2, bdiag, ones1, out):
    nc = tc.nc
    mm = nc.tensor.matmul

    with (
        tc.tile_pool(name="cst", bufs=1) as cst,
        tc.tile_pool(name="pers4", bufs=1) as pers4,
        tc.tile_pool(name="drp", bufs=1, space="DRAM") as drp,
    ):
        # ---------------- constants ----------------
        hsc2_sb = cst.tile([2, FB], FP, tag="hsc2")
        nc.sync.dma_start(out=hsc2_sb[:], in_=hsc2)
        bdiag_sb = cst.tile([P, 2], BF, tag="bdiag")
        nc.sync.dma_start(out=bdiag_sb[:], in_=bdiag)
        ones_sb = cst.tile([P, 1], BF, tag="ones1")
        nc.sync.dma_start(out=ones_sb[:], in_=ones1)
        cond_sb = cst.tile([P, 4], FP, tag="cond")
        nc.sync.dma_start(out=cond_sb[:], in_=cond)
        gain_sb = cst.tile([P, DCH], FP, tag="gain")
        inv_rms = cst.tile([P, TT], FP, tag="invrms")
        sums_sb = cst.tile([P, TT], FP, tag="sums")
        denom_sb = cst.tile([NH, NQ], BF, tag="denom")

        # persistent through attention
        qT_bf = pers4.tile([P, FB, NQ], BF, tag="qT")
        kT_bf = pers4.tile([P, FB, L], BF, tag="kT")
        v_ext = pers4.tile([P, TT, NH, E + 1], BF, tag="vext")
        nc.vector.memset(v_ext[:], 1.0)  # ones col; data cells overwritten

        with tc.tile_pool(name="xtp", bufs=1) as xtp:
            xT_bf = xtp.tile([P, DCH, L], BF, tag="xT_bf")

            # ------------- stage 0/1: gain, x load+cast, inv_rms -------------
            with (
                tc.tile_pool(name="nwload", bufs=1) as nwload,
                tc.tile_pool(name="xload", bufs=2) as xload,
                tc.tile_pool(name="xsqp", bufs=1) as xsqp,
                tc.tile_pool(name="tmp1", bufs=1) as tmp1,
                tc.tile_pool(name="ps_a", bufs=2, space="PSUM") as psa,
            ):
                normw_sb = nwload.tile([P, 4, D], FP, tag="normw")
                nc.sync.dma_start(out=normw_sb[:], in_=normwT.rearrange(
                    "(c p) d -> p c d", p=P))
                for blk in range(DCH):
                    ps_g = psa.tile([P, 1], FP, tag="psg")
                    for c in range(4):
                        mm(ps_g[:], normw_sb[:, c, blk * P:(blk + 1) * P],
                           cond_sb[:, c:c + 1], start=(c == 0), stop=(c == 3))
                    nc.vector.tensor_scalar_add(gain_sb[:, blk:blk + 1],
                                                ps_g[:], 1.0)

                xsq = xsqp.tile([P, DCH, L], BF, tag="xsq")
                for dc in range(DCH):
                    xf = xload.tile([P, L], FP, tag="xf")
                    nc.sync.dma_start(out=xf[:],
                                      in_=xT[dc * P:(dc + 1) * P, :])
                    nc.vector.tensor_copy(out=xT_bf[:, dc, :], in_=xf[:])
                    nc.scalar.square(out=xsq[:, dc, :], in_=xT_bf[:, dc, :])

                for tt in range(TT):
                    ps_r = psa.tile([P, 1], FP, tag="psr")
                    for dc in range(DCH):
                        mm(ps_r[:], xsq[:, dc, tt * P:(tt + 1) * P],
                           ones_sb[:], start=(dc == 0), stop=(dc == DCH - 1))
                    nc.vector.tensor_copy(out=sums_sb[:, tt:tt + 1],
                                          in_=ps_r[:])
                # inv_rms = (sum/D + eps)^-0.5 = exp(-0.5*ln(sum/D + eps))
                lnt = tmp1.tile([P, TT], FP, tag="lnt")
                nc.scalar.activation(out=lnt[:], in_=sums_sb[:], func=AF.Ln,
                                     bias=EPS, scale=1.0 / D)
                nc.scalar.activation(out=inv_rms[:], in_=lnt[:], func=AF.Exp,
                                     scale=-0.5)

            # ------------- stage 2: qkv matmuls + per-fb cosine norms -------
            # Q and K thirds first (V last: it needs inv_rms). PSUM
            # evacuations ride on ACT (idle during this phase); the cosine
            # norms for each feature block are pipelined right behind its
            # QKV matmuls so nothing serializes at the phase boundary.
            with (
                tc.tile_pool(name="wthird", bufs=2) as wthird,
                tc.tile_pool(name="wload", bufs=2) as wload,
                tc.tile_pool(name="sqp", bufs=2) as sqp,
                tc.tile_pool(name="nstage", bufs=2) as nstage,
                tc.tile_pool(name="bcqk", bufs=1) as bcqk,
                tc.tile_pool(name="ps_qkv", bufs=4, space="PSUM") as psqkv,
                tc.tile_pool(name="ps_nrm", bufs=2, space="PSUM") as psn,
            ):
                def load_wthird(third):
                    w_bf = wthird.tile([P, DCH, D], BF, tag="wt")
                    for dc in range(DCH):
                        wf = wload.tile([P, D], FP, tag="wf")
                        nc.sync.dma_start(
                            out=wf[:],
                            in_=qkvwT[dc * P:(dc + 1) * P,
                                      third * D:(third + 1) * D])
                        nc.vector.tensor_scalar_mul(w_bf[:, dc, :], wf[:],
                                                    gain_sb[:, dc:dc + 1])
                    return w_bf

                def fb_norm(fb, t_bf, n_tok, rec_d, use_s):
                    """cosine-norm factors for the 2 heads of block fb of
                    t_bf ([P, FB, n_tok]) -> bf16 rows in DRAM rec_d, then
                    broadcast-multiply t_bf[:, fb, :] in place."""
                    sq = sqp.tile([P, n_tok], BF, tag=f"sq{n_tok}")
                    nc.vector.tensor_mul(sq[:], t_bf[:, fb, :],
                                         t_bf[:, fb, :])
                    st = nstage.tile([2, n_tok], BF, tag=f"st{n_tok}")
                    for g in range(n_tok // 1024):
                        ps_n = psn.tile([2, 1024], FP, tag="psn")
                        for c in range(2):
                            mm(ps_n[:, c * 512:(c + 1) * 512], bdiag_sb[:],
                               sq[:, (g * 2 + c) * 512:(g * 2 + c + 1) * 512],
                               start=True, stop=True)
                        nc.vector.tensor_copy(
                            out=st[:, g * 1024:(g + 1) * 1024], in_=ps_n[:])
                    # rec = s * (st + eps)^-0.5 = s * exp(-0.5*ln(st+eps))
                    nc.scalar.activation(out=st[:], in_=st[:], func=AF.Ln,
                                         bias=EPS)
                    rec = nstage.tile([2, n_tok], BF, tag=f"rec{n_tok}")
                    nc.scalar.activation(out=rec[:], in_=st[:], func=AF.Exp,
                                         scale=-0.5)
                    if use_s:
                        nc.vector.tensor_scalar_mul(rec[:], rec[:],
                                                    hsc2_sb[:, fb:fb + 1])
                    nc.sync.dma_start(out=rec_d[2 * fb:2 * fb + 2, :],
                                      in_=rec[:])
                    bc = bcqk.tile([P, n_tok], BF, tag=f"bc{n_tok}")
                    nc.sync.dma_start(
                        out=bc[0:E, :],
                        in_=_bcast_rows(rec_d[2 * fb:2 * fb + 1, :], E,
                                        n_tok))
                    nc.sync.dma_start(
                        out=bc[E:P, :],
                        in_=_bcast_rows(rec_d[2 * fb + 1:2 * fb + 2, :], E,
                                        n_tok))
                    nc.vector.tensor_mul(t_bf[:, fb, :], t_bf[:, fb, :],
                                         bc[:])

                recq_d = drp.tile([NH, NQ], BF, tag="recq_d")
                reck_d = drp.tile([NH, L], BF, tag="reck_d")

                # Q: qT[feat, q] for this core's query half (= columns 0:NQ)
                wq = load_wthird(0)
                for fb in range(FB):
                    for qc in range(NQC):
                        ps = psqkv.tile([P, 512], FP, tag="psqkv")
                        for dc in range(DCH):
                            mm(ps[:], wq[:, dc, fb * P:(fb + 1) * P],
                               xT_bf[:, dc, qc * 512:(qc + 1) * 512],
                               start=(dc == 0), stop=(dc == DCH - 1))
                        nc.vector.tensor_copy(
                            out=qT_bf[:, fb, qc * 512:(qc + 1) * 512],
                            in_=ps[:])
                    fb_norm(fb, qT_bf, NQ, recq_d, use_s=True)

                # K: kT[feat, k] over all L tokens
                wk = load_wthird(1)
                for fb in range(FB):
                    for kc in range(LC):
                        ps = psqkv.tile([P, 512], FP, tag="psqkv")
                        for dc in range(DCH):
                            mm(ps[:], wk[:, dc, fb * P:(fb + 1) * P],
                               xT_bf[:, dc, kc * 512:(kc + 1) * 512],
                               start=(dc == 0), stop=(dc == DCH - 1))
                        nc.vector.tensor_copy(
                            out=kT_bf[:, fb, kc * 512:(kc + 1) * 512],
                            in_=ps[:])
                    fb_norm(fb, kT_bf, L, reck_d, use_s=False)

                # V: v[tok, feat] natural, scaled by inv_rms, into v_ext
                wv = load_wthird(2)
                for tt in range(TT):
                    for vc in range(2):
                        ps = psqkv.tile([P, 512], FP, tag="psqkv")
                        for dc in range(DCH):
                            mm(ps[:], xT_bf[:, dc, tt * P:(tt + 1) * P],
                               wv[:, dc, vc * 512:(vc + 1) * 512],
                               start=(dc == 0), stop=(dc == DCH - 1))
                        nc.vector.tensor_scalar_mul(
                            v_ext[:, tt, vc * 8:(vc + 1) * 8, 0:E],
                            ps[:].rearrange("p (h e) -> p h e", e=E),
                            inv_rms[:, tt:tt + 1])

        # ---------------- stage 4: attention ----------------
        with (
            tc.tile_pool(name="o4p", bufs=1) as o4p,
            tc.tile_pool(name="owload", bufs=2) as owload,
            tc.tile_pool(name="attnp", bufs=4) as attnp,
            tc.tile_pool(name="ottp", bufs=3) as ottp,
            tc.tile_pool(name="bcdp", bufs=2) as bcdp,
        ):
            outw_bf = o4p.tile([P, FB, D], BF, tag="outw")
            for dc in range(DCH):
                owf = owload.tile([P, D], FP, tag="owf")
                nc.sync.dma_start(out=owf[:],
                                  in_=outwT[dc * P:(dc + 1) * P, :])
                nc.vector.tensor_copy(out=outw_bf[:, dc, :], in_=owf[:])
            oT_sb = o4p.tile([P, FB, NQ], BF, tag="oT")

            groups = [list(range(g * KTG, min(TT, (g + 1) * KTG)))
                      for g in range((TT + KTG - 1) // KTG)]
            NG = len(groups)
            with (
                tc.tile_pool(name="ps_sa", bufs=1, space="PSUM") as pssa,
                tc.tile_pool(name="ps_sb", bufs=1, space="PSUM") as pssb,
                tc.tile_pool(name="ps_oa", bufs=1, space="PSUM") as psoa,
                tc.tile_pool(name="ps_ob", bufs=1, space="PSUM") as psob,
            ):
                # Head-paired attention: the two heads of a feature block
                # live at partitions 0-63 / 64-127, so their K=64 scores
                # matmuls (interleaved per key tile) occupy disjoint row
                # groups of the PE array and run concurrently. Scores+exp
                # run LAG task-slots ahead of attn@v in one flat global
                # pipeline, so neither PE nor ACT stalls at boundaries.
                state = {}

                def sc_task(fb, qc, g):
                    kts = groups[g]
                    psA = pssa.tile([P, KTG * 512], FP, tag="psA",
                                    name=f"psA_{fb}_{qc}_{g}")
                    psB = pssb.tile([P, KTG * 512], FP, tag="psB",
                                    name=f"psB_{fb}_{qc}_{g}")
                    for j, kt in enumerate(kts):
                        for hh, ps in ((0, psA), (1, psB)):
                            pb = E * hh
                            mm(ps[:, j * 512:(j + 1) * 512],
                               kT_bf[pb:pb + E, fb, kt * P:(kt + 1) * P],
                               qT_bf[pb:pb + E, fb, qc * 512:(qc + 1) * 512],
                               start=True, stop=True)
                    for hh, ps in ((0, psA), (1, psB)):
                        attg = attnp.tile([P, KTG * 512], BF, tag="attg",
                                          name=f"attg_{fb}_{qc}_{g}_{hh}")
                        state[(fb, qc, g, hh)] = attg
                        nc.scalar.activation(out=attg[:, :len(kts) * 512],
                                             in_=ps[:, :len(kts) * 512],
                                             func=AF.Exp)

                def av_task(fb, qc, g):
                    if g == 0:
                        state[(fb, qc, 0)] = psoa.tile(
                            [E + 1, 512], FP, tag="psoA",
                            name=f"psoA_{fb}_{qc}")
                        state[(fb, qc, 1)] = psob.tile(
                            [E + 1, 512], FP, tag="psoB",
                            name=f"psoB_{fb}_{qc}")
                    for j, kt in enumerate(groups[g]):
                        for hh in (0, 1):
                            attg = state[(fb, qc, g, hh)]
                            mm(state[(fb, qc, hh)][:],
                               v_ext[:, kt, 2 * fb + hh, :],
                               attg[:, j * 512:(j + 1) * 512],
                               start=(kt == 0), stop=(kt == TT - 1))
                    for hh in (0, 1):
                        del state[(fb, qc, g, hh)]
                    if g == NG - 1:
                        for hh in (0, 1):
                            pso = state.pop((fb, qc, hh))
                            pb = E * hh
                            h = 2 * fb + hh
                            ot = ottp.tile([E + 1, 512], BF, tag="ottmp")
                            nc.vector.tensor_copy(out=ot[:], in_=pso[:])
                            nc.sync.dma_start(
                                out=oT_sb[pb:pb + E, fb,
                                          qc * 512:(qc + 1) * 512],
                                in_=ot[0:E, :])
                            nc.sync.dma_start(
                                out=denom_sb[h:h + 1,
                                             qc * 512:(qc + 1) * 512],
                                in_=ot[E:E + 1, :])

                flat = [(fb, qc, g) for fb in range(FB) for qc in range(NQC)
                        for g in range(NG)]
                LAG = 2
                for idx in range(len(flat) + LAG):
                    if idx < len(flat):
                        sc_task(*flat[idx])
                    if idx >= LAG:
                        av_task(*flat[idx - LAG])

            # normalize oT by 1/denom
            ld = ottp.tile([NH, NQ], FP, tag="ld")
            nc.scalar.activation(out=ld[:], in_=denom_sb[:], func=AF.Ln)
            recd_bf = ottp.tile([NH, NQ], BF, tag="recd")
            nc.scalar.activation(out=recd_bf[:], in_=ld[:], func=AF.Exp,
                                 scale=-1.0)
            recd_d = drp.tile([NH, NQ], BF, tag="recd_d")
            nc.sync.dma_start(out=recd_d[:], in_=recd_bf[:])
            for h in range(NH):
                fb, hh = h // 2, h % 2
                pb = E * hh
                bcd = bcdp.tile([P, NQ], BF, tag="bcd")
                nc.sync.dma_start(
                    out=bcd[pb:pb + E, :],
                    in_=_bcast_rows(recd_d[h:h + 1, :], E, NQ))
                nc.vector.tensor_mul(oT_sb[pb:pb + E, fb, :],
                                     oT_sb[pb:pb + E, fb, :],
                                     bcd[pb:pb + E, :])

            # ------------- stage 5: out projection + residual -------------
            with (
                tc.tile_pool(name="skipp", bufs=2) as skipp,
                tc.tile_pool(name="outp", bufs=2) as outp,
                tc.tile_pool(name="ps_out", bufs=4, space="PSUM") as psout,
            ):
                for tq in range(QTT):
                    xs = skipp.tile([P, D], FP, tag="xs")
                    nc.sync.dma_start(out=xs[:],
                                      in_=xskip[tq * P:(tq + 1) * P, :])
                    osb = outp.tile([P, D], FP, tag="osb")
                    for dc2 in range(2):
                        ps = psout.tile([P, 512], FP, tag="psout")
                        for fb in range(FB):
                            mm(ps[:], oT_sb[:, fb, tq * P:(tq + 1) * P],
                               outw_bf[:, fb, dc2 * 512:(dc2 + 1) * 512],
                               start=(fb == 0), stop=(fb == FB - 1))
                        nc.vector.tensor_add(
                            osb[:, dc2 * 512:(dc2 + 1) * 512], ps[:],
                            xs[:, dc2 * 512:(dc2 + 1) * 512])
                    nc.sync.dma_start(out=out[tq * P:(tq + 1) * P, :],
                                      in_=osb[:])


def _make_in_maps(x, cond_norm, norm_w, qkv_w, head_scale, out_w):
    qkvwT = np.ascontiguousarray(qkv_w.T)
    outwT = np.ascontiguousarray(out_w.T)
    normwT = np.ascontiguousarray(norm_w.T)
    # hsc2[hh, fb] = head_scale[2*fb + hh]
    hsc2 = np.ascontiguousarray(head_scale.reshape(FB, 2).T)
    bd = np.zeros((P, 2), dtype=ml_dtypes.bfloat16)
    bd[0:E, 0] = 1.0
    bd[E:P, 1] = 1.0
    ones1 = np.ones((P, 1), dtype=ml_dtypes.bfloat16)

    in_maps = []
    for core in range(8):
        b, half = core // 2, core % 2
        xTb = x[b].T  # [D, L]
        if half == 0:
            xTr = np.ascontiguousarray(xTb)
        else:
            # rotate so this core's query half occupies columns 0..NQ-1
            xTr = np.ascontiguousarray(
                np.concatenate([xTb[:, NQ:], xTb[:, :NQ]], axis=1))
        in_maps.append({
            "xT": xTr,
            "xskip": np.ascontiguousarray(x[b, half * NQ:(half + 1) * NQ, :]),
            "qkvwT": qkvwT,
            "outwT": outwT,
            "normwT": normwT,
            "cond": np.ascontiguousarray(cond_norm[b].reshape(4, P).T),
            "hsc2": hsc2,
            "bdiag": bd,
            "ones1": ones1,
        })
    return in_maps


def get_nc():
    if "nc" not in _CACHED:
        _CACHED["nc"] = _build_nc()
    return _CACHED["nc"]


def run(inputs, trace=False):
    """Returns (full_output, BassKernelResults)."""
    x = np.asarray(inputs["x"], dtype=np.float32)
    in_maps = _make_in_maps(
        x,
        np.asarray(inputs["cond_norm"], dtype=np.float32),
        np.asarray(inputs["norm_w"], dtype=np.float32),
        np.asarray(inputs["qkv_w"], dtype=np.float32),
        np.asarray(inputs["head_scale"], dtype=np.float32),
        np.asarray(inputs["out_w"], dtype=np.float32),
    )
    nc = get_nc()
    res = run_bass_kernel_spmd(nc, in_maps, core_ids=list(range(8)),
                               trace=trace)
    full = np.empty((N_B, L, D), dtype=np.float32)
    for core in range(8):
        b, half = core // 2, core % 2
        full[b, half * NQ:(half + 1) * NQ, :] = res.results[core]["out"]
    return full, res


def kernel(**inputs) -> np.ndarray:
    full, _ = run(inputs, trace=False)
    return full
